# revision 17
# baseline (speedup 1.0000x reference)
"""Multi-head causal attention (b=4, t=2048, k=1024, h=16) on 8 Trainium2 cores.

Sharding: core c = (batch b=c//2, head-group g=c%2). Each core computes one
batch x 8 heads; the two half-head partial outputs per batch are summed on
host.

Per-core kernel (bf16 matmul paths, fp32 PSUM), fully software-pipelined so
the tensor engine never idles (idle gaps also drop the PE clock 2.4->1.2GHz):
  - Q/K projections write per-head zero-PADDED tiles qtp/ktp[128, h, t]
    (head data in partitions 0:64, zeros in 64:128): the PE runs at half
    rate when contraction or stationary dims are < 128. Odd heads are
    placed via SBUF->SBUF DMA partition shift.
  - V is stored [128, ki, h, 128]: cols 0:64 = V, col 64 = ones (softmax
    denominator via the augmented PV matmul), 65:128 zeros.
  - Emission order interleaves projection matmul groups between attention
    chunks: K0/Q0 first, V tiles just-in-time inside head 0, K/Q tile g+1
    as filler during heads 2g-2..2g-1, output projection (phase C) m-tiles
    as filler inside head 7 as their ot columns complete.
  - Softmax: exp on ACT (bf16 out); reciprocal as exp(-ln d) on ACT (both
    funcs forced into one activation table => a single table load);
    denominator broadcast via a [1,128] ones matmul; normalize on DVE.
"""
import sys

sys.path.insert(0, "/opt/trn_rl_repo")

import numpy as np
import ml_dtypes

import concourse.bass as bass
import concourse.mybir as mybir
import concourse.tile as tile
from concourse import bacc
from concourse.bass_utils import run_bass_kernel_spmd
from concourse.masks import make_upper_triangular

# Force every ACT func (Exp/Ln/Copy) onto the one table that contains them
# all, so the table-load pass emits a single load instead of ping-ponging
# between exp-only and ln-only tables (1.3us per reload). Indices into
# act_info.json are preserved; only the candidacy of the other tables is
# hidden from the chooser.
_ORIG_GET_TABLES = bacc.get_activation_tables


def _single_table_get_activation_tables(arch):
    tabs = _ORIG_GET_TABLES(arch)
    if "natural_log_exp_and_others" not in tabs:
        return tabs
    return {
        name: (funcs if name == "natural_log_exp_and_others" else set())
        for name, funcs in tabs.items()
    }


bacc.get_activation_tables = _single_table_get_activation_tables

F32 = mybir.dt.float32
F32R = mybir.dt.float32r
BF16 = mybir.dt.bfloat16
EXP = mybir.ActivationFunctionType.Exp
LN = mybir.ActivationFunctionType.Ln

B, T, KD, NH, HS = 4, 2048, 1024, 16, 64
NCORES = 8


def build_nc(t=T, dl=512, hl=8, kd=KD):
    """One core's program: x.T [kd,t], per-group weights, partial out [t,kd]."""
    nk = kd // 128       # contraction tiles for projections
    mt = t // 128        # t tiles (also k-position tiles in attention)
    dt = dl // 128       # local-dim tiles
    nqc = t // 512       # q chunks
    scale = 1.0 / float(np.sqrt(kd))

    nc = bacc.Bacc("TRN2", target_bir_lowering=False, debug=False, num_devices=NCORES)
    xt_d = nc.dram_tensor("xt", [kd, t], BF16, kind="ExternalInput")
    wq_d = nc.dram_tensor("wq", [kd, dl], BF16, kind="ExternalInput")
    wk_d = nc.dram_tensor("wk", [kd, dl], BF16, kind="ExternalInput")
    wv_d = nc.dram_tensor("wv", [kd, dl], BF16, kind="ExternalInput")
    wo_d = nc.dram_tensor("wo", [dl, kd], BF16, kind="ExternalInput")
    out_d = nc.dram_tensor("out", [t, kd], F32, kind="ExternalOutput")
    out2_d = nc.dram_tensor("out2", [t, kd], F32, kind="ExternalOutput")

    with tile.TileContext(nc) as tc:
        with (
            tc.tile_pool(name="persist", bufs=1) as pp,
            tc.tile_pool(name="misc", bufs=1) as mp,
            tc.tile_pool(name="pa", bufs=1) as pa,
            tc.tile_pool(name="past", bufs=4) as past,
            tc.tile_pool(name="pbe", bufs=4) as pbe,
            tc.tile_pool(name="pbm", bufs=2) as pbm,
            tc.tile_pool(name="pco", bufs=2) as pco,
            tc.tile_pool(name="ps", bufs=1, space="PSUM") as ps,
        ):
            qtp = pp.tile([128, hl, t], BF16)   # per-head padded Q^T
            ktp = pp.tile([128, hl, t], BF16)   # per-head padded K^T
            v_s = pp.tile([128, mt, hl, 128], BF16)  # V | ones | zeros
            ot_s = pp.tile([128, dt, t], BF16)
            wo_s = pp.tile([128, dt, kd], BF16)
            mask_t = mp.tile([128, 128], BF16)
            ones_t = mp.tile([1, 128], BF16)
            ones128 = mp.tile([128, 128], BF16)
            xt_s = pa.tile([128, nk, t], BF16)
            wq_s = pa.tile([128, nk, dl], BF16)
            wk_s = pa.tile([128, nk, dl], BF16)
            wv_s = pa.tile([128, nk, dl], BF16)

            # ---------------- input DMAs (priority order) ----------------
            nc.sync.dma_start(
                wk_s[:, :, :], wk_d[:, :].rearrange("(n p) d -> p n d", p=128)
            )
            xt_r = xt_d[:, :].rearrange("(n p) t -> p n t", p=128)
            for k in range(nk):
                nc.sync.dma_start(xt_s[:, k, :], xt_r[:, k, :])
            nc.sync.dma_start(
                wq_s[:, :, :], wq_d[:, :].rearrange("(n p) d -> p n d", p=128)
            )
            nc.sync.dma_start(
                wv_s[:, :, :], wv_d[:, :].rearrange("(n p) d -> p n d", p=128)
            )
            nc.sync.dma_start(
                wo_s[:, :, :], wo_d[:, :].rearrange("(n p) o -> p n o", p=128)
            )

            # ---------------- constants + padding zeros ----------------
            make_upper_triangular(nc, mask_t[:, :], val=1.0, diag=True)
            nc.vector.memset(ones128[:, :], 1.0)
            nc.scalar.copy(ones_t[:, :], ones128[0:1, :])
            nc.scalar.copy(
                v_s[:, :, :, 64],
                ones128[:, 0 : mt * hl].rearrange("p (m h) -> p m h", m=mt),
            )
            # fine-grained zeroing in first-use order: head h's pad gates
            # only that head's scores; v pad per ki gates only that PV step.
            nc.gpsimd.memset(qtp[64:128, 0, :], 0.0)
            nc.gpsimd.memset(ktp[64:128, 0, :], 0.0)
            for ki in range(4):
                nc.gpsimd.memset(v_s[:, ki, :, 65:128], 0.0)
            for h in range(1, hl):
                nc.gpsimd.memset(qtp[64:128, h, :], 0.0)
                nc.gpsimd.memset(ktp[64:128, h, :], 0.0)
                if h < 4:
                    for ki in range(4 * h, 4 * h + 4):
                        nc.gpsimd.memset(v_s[:, ki, :, 65:128], 0.0)

            # ---------------- emission helpers ----------------
            def proj_v(m):
                psv = ps.tile([128, dl], F32, name=f"psv{m}", tag="proj", bufs=2)
                for k in range(nk):
                    nc.tensor.matmul(
                        psv[:, :],
                        xt_s[:, k, 128 * m : 128 * m + 128],
                        wv_s[:, k, :],
                        start=(k == 0),
                        stop=(k == nk - 1),
                    )
                nc.vector.tensor_copy(
                    v_s[:, m, :, 0:64],
                    psv[:, :].rearrange("p (h d) -> p h d", h=hl),
                )

            def proj_qk(w_s, o_s, pfx, m, n):
                # one 512-col group of Q or K dtile m (heads 2m, 2m+1)
                cols = slice(512 * n, 512 * n + 512)
                psq = ps.tile(
                    [128, 512], F32, name=f"ps{pfx}{m}_{n}", tag="proj", bufs=2
                )
                for k in range(nk):
                    nc.tensor.matmul(
                        psq[:, :],
                        w_s[:, k, 128 * m : 128 * m + 128],
                        xt_s[:, k, cols],
                        start=(k == 0),
                        stop=(k == nk - 1),
                    )
                nc.vector.tensor_copy(o_s[0:64, 2 * m, cols], psq[0:64, :])
                st = past.tile(
                    [128, 512], BF16, name=f"st{pfx}{m}_{n}", tag="stage"
                )
                nc.vector.tensor_copy(st[64:128, :], psq[64:128, :])
                nc.sync.dma_start(o_s[0:64, 2 * m + 1, cols], st[64:128, :])

            def score_chunk(h, ki, qc):
                q0 = 128 * ki
                off = max(q0, 512 * qc)
                w = 512 * (qc + 1) - off
                stp = ps.tile(
                    [128, 512], F32, name=f"st{h}_{ki}_{qc}", tag="st", bufs=2
                )
                nc.tensor.matmul(
                    stp[:, :w],
                    ktp[:, h, q0 : q0 + 128],
                    qtp[:, h, off : off + w],
                    start=True,
                    stop=True,
                )
                ex = pbe.tile(
                    [128, 512], BF16, name=f"ex{h}_{ki}_{qc}", tag="exp"
                )
                nc.scalar.activation(ex[:, :w], stp[:, :w], EXP, scale=scale)
                if off == q0:
                    nc.vector.tensor_mul(ex[:, 0:128], ex[:, 0:128], mask_t[:, :])
                return ex

            def pv_chunk(h, ki, qc, otp, ex):
                q0 = 128 * ki
                off = max(q0, 512 * qc)
                w = 512 * (qc + 1) - off
                co = off - 512 * qc
                nc.tensor.matmul(
                    otp[qc][:, co : co + w],
                    v_s[:, ki, h, :],
                    ex[:, :w],
                    start=(ki == 0),
                    stop=(ki == 4 * qc + 3),
                )

            def attn_ki(h, ki, otp):
                for qc in range(128 * ki // 512, nqc):
                    ex = score_chunk(h, ki, qc)
                    pv_chunk(h, ki, qc, otp, ex)

            def normalize(h, qc, otp):
                mh, ph = h // 2, 64 * (h % 2)
                # 1/d = exp(-ln d) on ACT (no table swap), broadcast across
                # 64 partitions with a ones[1,128] matmul, multiply on DVE.
                rec = pbm.tile([1, 512], F32, name=f"rc{h}_{qc}", tag="rec")
                recb = pbm.tile([1, 512], BF16, name=f"rb{h}_{qc}", tag="recb")
                with nc.allow_low_precision(reason="softmax denom"):
                    nc.scalar.activation(rec[:, :], otp[qc][64:65, :], LN)
                    nc.scalar.activation(recb[:, :], rec[:, :], EXP, scale=-1.0)
                bc = ps.tile([128, 512], F32, name=f"bc{h}_{qc}", tag="st", bufs=2)
                nc.tensor.matmul(
                    bc[:, :], ones_t[:, :], recb[:, :], start=True, stop=True
                )
                cols = slice(512 * qc, 512 * qc + 512)
                with nc.allow_low_precision(reason="softmax normalize"):
                    if ph == 0:
                        dst = ot_s[0:64, mh, cols]
                        nc.vector.tensor_copy(dst, otp[qc][0:64, :])
                        nc.vector.tensor_mul(dst, dst, bc[0:64, :])
                    else:
                        sc = pbm.tile(
                            [64, 512], BF16, name=f"sc{h}_{qc}", tag="scr"
                        )
                        nc.vector.tensor_copy(sc[:, :], otp[qc][0:64, :])
                        nc.vector.tensor_mul(sc[:, :], sc[:, :], bc[0:64, :])
                        nc.sync.dma_start(ot_s[64:128, mh, cols], sc[:, :])

            def phasec(m, ks, dst):
                ob = pco.tile([128, kd], F32, name=f"ob{ks[0]}_{m}", tag="ob")
                for n in range(kd // 512):
                    pso = ps.tile(
                        [128, 512], F32, name=f"pso{ks[0]}_{m}_{n}", tag="proj",
                        bufs=2,
                    )
                    for j, k in enumerate(ks):
                        nc.tensor.matmul(
                            pso[:, :],
                            ot_s[:, k, 128 * m : 128 * m + 128],
                            wo_s[:, k, 512 * n : 512 * n + 512],
                            start=(j == 0),
                            stop=(j == len(ks) - 1),
                        )
                    nc.vector.tensor_copy(ob[:, 512 * n : 512 * n + 512], pso[:, :])
                nc.sync.dma_start(dst[128 * m : 128 * m + 128, :], ob[:, :])

            # ---------------- emission schedule ----------------
            # Head 0 runs a special prologue: only K0g0+Q0g0 gate the first
            # scores chunk, so the ACT exp chain starts as soon as x/wk/wq
            # arrive instead of after all 8 projection groups.
            otp0 = [
                ps.tile([128, 512], F32, name=f"otp0_{qc}", tag="ot", bufs=4)
                for qc in range(nqc)
            ]
            proj_qk(wk_s, ktp, "k", 0, 0)
            ex0 = []
            for qc in range(nqc):
                proj_qk(wq_s, qtp, "q", 0, qc)
                ex0.append(score_chunk(0, 0, qc))
            proj_v(0)
            for qc in range(nqc):
                pv_chunk(0, 0, qc, otp0, ex0[qc])

            # filler projection groups: heads 2g,2g+1 need K/Q tile g,
            # emitted during heads 2g-2 and 2g-1; phase C first half
            # (heads 0..3, k-tiles 0,1) fills heads 4..6 into out2.
            filler = {h: [] for h in range(hl)}
            filler[0] = [
                (lambda nn=n: proj_qk(wk_s, ktp, "k", 0, nn)) for n in (1, 2, 3)
            ]
            for g in range(1, dt):
                filler[2 * g - 2] += [
                    (lambda gg=g, nn=n: proj_qk(wk_s, ktp, "k", gg, nn))
                    for n in range(nqc)
                ]
                filler[2 * g - 1] += [
                    (lambda gg=g, nn=n: proj_qk(wq_s, qtp, "q", gg, nn))
                    for n in range(nqc)
                ]
            filler[4] += [(lambda mm=m: phasec(mm, (0, 1), out2_d)) for m in range(4)]
            filler[5] += [(lambda mm=m: phasec(mm, (0, 1), out2_d)) for m in range(4, 8)]
            filler[6] = [(lambda mm=m: phasec(mm, (0, 1), out2_d)) for m in range(8, mt)]

            cready = []   # phase C second-half m-tiles (during h7)
            for h in range(hl):
                if h == 0:
                    otp = otp0
                else:
                    otp = [
                        ps.tile(
                            [128, 512], F32, name=f"otp{h}_{qc}", tag="ot", bufs=4
                        )
                        for qc in range(nqc)
                    ]
                fill = filler[h]
                fi = 0
                for ki in range(1 if h == 0 else 0, mt):
                    if h == 0:
                        proj_v(ki)
                    attn_ki(h, ki, otp)
                    want = (ki + 1) * len(fill) // mt
                    while fi < want:
                        fill[fi]()
                        fi += 1
                    if ki % 4 == 3:
                        qc = ki // 4
                        normalize(h, qc, otp)
                        if h == hl - 1:
                            cready += range(4 * qc, 4 * qc + 4)
                    if h == hl - 1 and cready:
                        phasec(cready.pop(0), (2, 3), out_d)
                while fi < len(fill):
                    fill[fi]()
                    fi += 1
            while cready:
                phasec(cready.pop(0), (2, 3), out_d)

    nc.finalize()
    return nc


_NC_CACHE = {}


def _get_nc(key=(T, 512, 8, KD)):
    if key not in _NC_CACHE:
        _NC_CACHE[key] = build_nc(*key)
    return _NC_CACHE[key]


def make_in_maps(x, Wq, Wk, Wv, Wo, dl=512):
    in_maps = []
    for c in range(NCORES):
        b, g = c // 2, c % 2
        rows = slice(dl * g, dl * (g + 1))
        in_maps.append(
            {
                "xt": np.ascontiguousarray(x[b].T).astype(ml_dtypes.bfloat16),
                "wq": np.ascontiguousarray(Wq[rows, :].T).astype(ml_dtypes.bfloat16),
                "wk": np.ascontiguousarray(Wk[rows, :].T).astype(ml_dtypes.bfloat16),
                "wv": np.ascontiguousarray(Wv[rows, :].T).astype(ml_dtypes.bfloat16),
                "wo": np.ascontiguousarray(Wo[:, rows].T).astype(ml_dtypes.bfloat16),
            }
        )
    return in_maps


def run_spmd(x, Wq, Wk, Wv, Wo, trace=False):
    nc = _get_nc()
    in_maps = make_in_maps(x, Wq, Wk, Wv, Wo)
    res = run_bass_kernel_spmd(nc, in_maps, list(range(NCORES)), trace=trace)
    outs = [res.results[c]["out"] for c in range(NCORES)]
    outs2 = [res.results[c]["out2"] for c in range(NCORES)]
    final = np.stack(
        [
            outs[2 * b] + outs2[2 * b] + outs[2 * b + 1] + outs2[2 * b + 1]
            for b in range(B)
        ]
    )
    return final.astype(np.float32), res


def kernel(x, Wq, Wk, Wv, Wo):
    x = np.asarray(x, dtype=np.float32)
    Wq = np.asarray(Wq, dtype=np.float32)
    Wk = np.asarray(Wk, dtype=np.float32)
    Wv = np.asarray(Wv, dtype=np.float32)
    out, _ = run_spmd(x, Wq, Wk, Wv, np.asarray(Wo, dtype=np.float32))
    return out


# revision 19
# speedup vs baseline: 1.0471x; 1.0471x over previous
"""Multi-head causal attention (b=4, t=2048, k=1024, h=16) on 8 Trainium2 cores.

Sharding: core c = (batch b=c//2, head-group g=c%2). Each core computes one
batch x 8 heads; the two half-head partial outputs per batch are summed on
host.

Per-core kernel (bf16 matmul paths, fp32 PSUM), fully software-pipelined so
the tensor engine never idles (idle gaps also drop the PE clock 2.4->1.2GHz):
  - Q/K projections write per-head zero-PADDED tiles qtp/ktp[128, h, t]
    (head data in partitions 0:64, zeros in 64:128): the PE runs at half
    rate when contraction or stationary dims are < 128. Odd heads are
    placed via SBUF->SBUF DMA partition shift.
  - V is stored [128, ki, h, 128]: cols 0:64 = V, col 64 = ones (softmax
    denominator via the augmented PV matmul), 65:128 zeros.
  - Emission order interleaves projection matmul groups between attention
    chunks: K0/Q0 first, V tiles just-in-time inside head 0, K/Q tile g+1
    as filler during heads 2g-2..2g-1, output projection (phase C) m-tiles
    as filler inside head 7 as their ot columns complete.
  - Softmax: exp on ACT (bf16 out); reciprocal as exp(-ln d) on ACT (both
    funcs forced into one activation table => a single table load);
    denominator broadcast via a [1,128] ones matmul; normalize on DVE.
"""
import sys

sys.path.insert(0, "/opt/trn_rl_repo")

import numpy as np
import ml_dtypes

import concourse.bass as bass
import concourse.mybir as mybir
import concourse.tile as tile
from concourse import bacc
from concourse.bass_utils import run_bass_kernel_spmd
from concourse.masks import make_upper_triangular

# Force every ACT func (Exp/Ln/Copy) onto the one table that contains them
# all, so the table-load pass emits a single load instead of ping-ponging
# between exp-only and ln-only tables (1.3us per reload). Indices into
# act_info.json are preserved; only the candidacy of the other tables is
# hidden from the chooser.
_ORIG_GET_TABLES = bacc.get_activation_tables


def _single_table_get_activation_tables(arch):
    tabs = _ORIG_GET_TABLES(arch)
    if "natural_log_exp_and_others" not in tabs:
        return tabs
    return {
        name: (funcs if name == "natural_log_exp_and_others" else set())
        for name, funcs in tabs.items()
    }


bacc.get_activation_tables = _single_table_get_activation_tables

F32 = mybir.dt.float32
F32R = mybir.dt.float32r
BF16 = mybir.dt.bfloat16
EXP = mybir.ActivationFunctionType.Exp
LN = mybir.ActivationFunctionType.Ln

B, T, KD, NH, HS = 4, 2048, 1024, 16, 64
NCORES = 8


def build_nc(t=T, dl=512, hl=8, kd=KD):
    """One core's program: x.T [kd,t], per-group weights, partial out [t,kd]."""
    nk = kd // 128       # contraction tiles for projections
    mt = t // 128        # t tiles (also k-position tiles in attention)
    dt = dl // 128       # local-dim tiles
    nqc = t // 512       # q chunks
    scale = 1.0 / float(np.sqrt(kd))

    nc = bacc.Bacc("TRN2", target_bir_lowering=False, debug=False, num_devices=NCORES)
    xt_d = nc.dram_tensor("xt", [kd, t], BF16, kind="ExternalInput")
    wq_d = nc.dram_tensor("wq", [kd, dl], BF16, kind="ExternalInput")
    wk_d = nc.dram_tensor("wk", [kd, dl], BF16, kind="ExternalInput")
    wv_d = nc.dram_tensor("wv", [kd, dl], BF16, kind="ExternalInput")
    wo_d = nc.dram_tensor("wo", [dl, kd], BF16, kind="ExternalInput")
    out_d = nc.dram_tensor("out", [t, kd], F32, kind="ExternalOutput")
    out2_d = nc.dram_tensor("out2", [t, kd], F32, kind="ExternalOutput")

    with tile.TileContext(nc) as tc:
        with (
            tc.tile_pool(name="persist", bufs=1) as pp,
            tc.tile_pool(name="misc", bufs=1) as mp,
            tc.tile_pool(name="pa", bufs=1) as pa,
            tc.tile_pool(name="past", bufs=4) as past,
            tc.tile_pool(name="pbe", bufs=4) as pbe,
            tc.tile_pool(name="pbm", bufs=2) as pbm,
            tc.tile_pool(name="pco", bufs=2) as pco,
            tc.tile_pool(name="ps", bufs=1, space="PSUM") as ps,
        ):
            qtp = pp.tile([128, hl, t], BF16)   # per-head padded Q^T
            ktp = pp.tile([128, hl, t], BF16)   # per-head padded K^T
            v_s = pp.tile([128, mt, hl, 128], BF16)  # V | ones | zeros
            ot_s = pp.tile([128, dt, t], BF16)
            wo_s = pp.tile([128, dt, kd], BF16)
            mask_t = mp.tile([128, 128], BF16)
            ones_t = mp.tile([1, 128], BF16)
            ones128 = mp.tile([128, 128], BF16)
            xt_s = pa.tile([128, nk, t], BF16)
            wq_s = pa.tile([128, nk, dl], BF16)
            wk_s = pa.tile([128, nk, dl], BF16)
            wv_s = pa.tile([128, nk, dl], BF16)

            # ---------------- input DMAs (priority order) ----------------
            nc.sync.dma_start(
                wk_s[:, :, :], wk_d[:, :].rearrange("(n p) d -> p n d", p=128)
            )
            xt_r = xt_d[:, :].rearrange("(n p) t -> p n t", p=128)
            for k in range(nk):
                nc.sync.dma_start(xt_s[:, k, :], xt_r[:, k, :])
            nc.sync.dma_start(
                wq_s[:, :, :], wq_d[:, :].rearrange("(n p) d -> p n d", p=128)
            )
            nc.sync.dma_start(
                wv_s[:, :, :], wv_d[:, :].rearrange("(n p) d -> p n d", p=128)
            )
            nc.sync.dma_start(
                wo_s[:, :, :], wo_d[:, :].rearrange("(n p) o -> p n o", p=128)
            )

            # ---------------- constants + padding zeros ----------------
            make_upper_triangular(nc, mask_t[:, :], val=1.0, diag=True)
            nc.vector.memset(ones128[:, :], 1.0)
            nc.scalar.copy(ones_t[:, :], ones128[0:1, :])
            nc.scalar.copy(
                v_s[:, :, :, 64],
                ones128[:, 0 : mt * hl].rearrange("p (m h) -> p m h", m=mt),
            )
            # fine-grained zeroing in first-use order: head h's pad gates
            # only that head's scores; v pad per ki gates only that PV step.
            nc.gpsimd.memset(qtp[64:128, 0, :], 0.0)
            nc.gpsimd.memset(ktp[64:128, 0, :], 0.0)
            for ki in range(4):
                nc.gpsimd.memset(v_s[:, ki, :, 65:128], 0.0)
            for h in range(1, hl):
                nc.gpsimd.memset(qtp[64:128, h, :], 0.0)
                nc.gpsimd.memset(ktp[64:128, h, :], 0.0)
                if h < 4:
                    for ki in range(4 * h, 4 * h + 4):
                        nc.gpsimd.memset(v_s[:, ki, :, 65:128], 0.0)

            # ---------------- emission helpers ----------------
            def proj_v(m):
                psv = ps.tile([128, dl], F32, name=f"psv{m}", tag="proj", bufs=2)
                for k in range(nk):
                    nc.tensor.matmul(
                        psv[:, :],
                        xt_s[:, k, 128 * m : 128 * m + 128],
                        wv_s[:, k, :],
                        start=(k == 0),
                        stop=(k == nk - 1),
                    )
                nc.vector.tensor_copy(
                    v_s[:, m, :, 0:64],
                    psv[:, :].rearrange("p (h d) -> p h d", h=hl),
                )

            def proj_qk(w_s, o_s, pfx, m, n):
                # one 512-col group of Q or K dtile m (heads 2m, 2m+1)
                cols = slice(512 * n, 512 * n + 512)
                psq = ps.tile(
                    [128, 512], F32, name=f"ps{pfx}{m}_{n}", tag="proj", bufs=2
                )
                for k in range(nk):
                    nc.tensor.matmul(
                        psq[:, :],
                        w_s[:, k, 128 * m : 128 * m + 128],
                        xt_s[:, k, cols],
                        start=(k == 0),
                        stop=(k == nk - 1),
                    )
                nc.vector.tensor_copy(o_s[0:64, 2 * m, cols], psq[0:64, :])
                st = past.tile(
                    [128, 512], BF16, name=f"st{pfx}{m}_{n}", tag="stage"
                )
                nc.vector.tensor_copy(st[64:128, :], psq[64:128, :])
                nc.sync.dma_start(o_s[0:64, 2 * m + 1, cols], st[64:128, :])

            def score_chunk(h, ki, qc):
                q0 = 128 * ki
                off = max(q0, 512 * qc)
                w = 512 * (qc + 1) - off
                stp = ps.tile(
                    [128, 512], F32, name=f"st{h}_{ki}_{qc}", tag="st", bufs=2
                )
                nc.tensor.matmul(
                    stp[:, :w],
                    ktp[:, h, q0 : q0 + 128],
                    qtp[:, h, off : off + w],
                    start=True,
                    stop=True,
                )
                ex = pbe.tile(
                    [128, 512], BF16, name=f"ex{h}_{ki}_{qc}", tag="exp"
                )
                nc.scalar.activation(ex[:, :w], stp[:, :w], EXP, scale=scale)
                if off == q0:
                    nc.vector.tensor_mul(ex[:, 0:128], ex[:, 0:128], mask_t[:, :])
                return ex

            def pv_chunk(h, ki, qc, otp, ex):
                q0 = 128 * ki
                off = max(q0, 512 * qc)
                w = 512 * (qc + 1) - off
                co = off - 512 * qc
                nc.tensor.matmul(
                    otp[qc][:, co : co + w],
                    v_s[:, ki, h, :],
                    ex[:, :w],
                    start=(ki == 0),
                    stop=(ki == 4 * qc + 3),
                )

            def attn_ki(h, ki, otp, mid=None):
                qcs = list(range(128 * ki // 512, nqc))
                exs = {}
                for qc in qcs:
                    exs[qc] = score_chunk(h, ki, qc)
                if mid is not None:
                    mid()
                for qc in qcs:
                    pv_chunk(h, ki, qc, otp, exs[qc])

            def normalize_rec(h, qc, otp):
                # 1/d = exp(-ln d) on ACT (no table swap). Emitted as soon
                # as otp[qc] stops accumulating; the tensor-side apply is
                # deferred so the bc matmul never waits on this ACT chain.
                rec = pbm.tile([1, 512], F32, name=f"rc{h}_{qc}", tag="rec")
                recb = pbm.tile([1, 512], BF16, name=f"rb{h}_{qc}", tag="recb")
                with nc.allow_low_precision(reason="softmax denom"):
                    nc.scalar.activation(rec[:, :], otp[qc][64:65, :], LN)
                    nc.scalar.activation(recb[:, :], rec[:, :], EXP, scale=-1.0)
                return recb

            def normalize_apply(h, qc, otp, recb):
                mh, ph = h // 2, 64 * (h % 2)
                bc = ps.tile([128, 512], F32, name=f"bc{h}_{qc}", tag="st", bufs=2)
                nc.tensor.matmul(
                    bc[:, :], ones_t[:, :], recb[:, :], start=True, stop=True
                )
                cols = slice(512 * qc, 512 * qc + 512)
                with nc.allow_low_precision(reason="softmax normalize"):
                    if ph == 0:
                        dst = ot_s[0:64, mh, cols]
                        nc.vector.tensor_copy(dst, otp[qc][0:64, :])
                        nc.vector.tensor_mul(dst, dst, bc[0:64, :])
                    else:
                        sc = pbm.tile(
                            [64, 512], BF16, name=f"sc{h}_{qc}", tag="scr"
                        )
                        nc.vector.tensor_copy(sc[:, :], otp[qc][0:64, :])
                        nc.vector.tensor_mul(sc[:, :], sc[:, :], bc[0:64, :])
                        nc.sync.dma_start(ot_s[64:128, mh, cols], sc[:, :])

            def phasec(m, ks, dst):
                ob = pco.tile([128, kd], F32, name=f"ob{ks[0]}_{m}", tag="ob")
                for n in range(kd // 512):
                    pso = ps.tile(
                        [128, 512], F32, name=f"pso{ks[0]}_{m}_{n}", tag="proj",
                        bufs=2,
                    )
                    for j, k in enumerate(ks):
                        nc.tensor.matmul(
                            pso[:, :],
                            ot_s[:, k, 128 * m : 128 * m + 128],
                            wo_s[:, k, 512 * n : 512 * n + 512],
                            start=(j == 0),
                            stop=(j == len(ks) - 1),
                        )
                    nc.vector.tensor_copy(ob[:, 512 * n : 512 * n + 512], pso[:, :])
                nc.sync.dma_start(dst[128 * m : 128 * m + 128, :], ob[:, :])

            # ---------------- emission schedule ----------------
            # Head 0 runs a special prologue: only K0g0+Q0g0 gate the first
            # scores chunk, so the ACT exp chain starts as soon as x/wk/wq
            # arrive instead of after all 8 projection groups.
            otp0 = [
                ps.tile([128, 512], F32, name=f"otp0_{qc}", tag="ot", bufs=4)
                for qc in range(nqc)
            ]
            proj_qk(wk_s, ktp, "k", 0, 0)
            ex0 = []
            for qc in range(nqc):
                proj_qk(wq_s, qtp, "q", 0, qc)
                ex0.append(score_chunk(0, 0, qc))
            proj_v(0)
            for qc in range(nqc):
                pv_chunk(0, 0, qc, otp0, ex0[qc])

            # filler projection groups: heads 2g,2g+1 need K/Q tile g,
            # emitted during heads 2g-2 and 2g-1; phase C first half
            # (heads 0..3, k-tiles 0,1) fills heads 4..6 into out2.
            filler = {h: [] for h in range(hl)}
            filler[0] = [
                (lambda nn=n: proj_qk(wk_s, ktp, "k", 0, nn)) for n in (1, 2, 3)
            ]
            for g in range(1, dt):
                filler[2 * g - 2] += [
                    (lambda gg=g, nn=n: proj_qk(wk_s, ktp, "k", gg, nn))
                    for n in range(nqc)
                ]
                filler[2 * g - 1] += [
                    (lambda gg=g, nn=n: proj_qk(wq_s, qtp, "q", gg, nn))
                    for n in range(nqc)
                ]
            filler[4] += [(lambda mm=m: phasec(mm, (0, 1), out2_d)) for m in range(4)]
            filler[5] += [(lambda mm=m: phasec(mm, (0, 1), out2_d)) for m in range(4, 8)]
            filler[6] = [(lambda mm=m: phasec(mm, (0, 1), out2_d)) for m in range(8, mt)]

            cready = []   # phase C second-half m-tiles (during h7)
            pending = []  # deferred normalize applies: (h, qc, otp, recb)

            def do_apply(args):
                ah, aqc = args[0], args[1]
                normalize_apply(*args)
                if ah == hl - 1:
                    cready.extend(range(4 * aqc, 4 * aqc + 4))
            for h in range(hl):
                if h == 0:
                    otp = otp0
                else:
                    otp = [
                        ps.tile(
                            [128, 512], F32, name=f"otp{h}_{qc}", tag="ot", bufs=4
                        )
                        for qc in range(nqc)
                    ]
                fill = filler[h]
                fi = 0
                for ki in range(1 if h == 0 else 0, mt):
                    if h == 0:
                        proj_v(ki)
                    # pop a deferred apply between scores and PVs so the bc
                    # matmul's ACT dependency has had time to resolve
                    mid = None
                    if pending and (
                        len(pending) > 1
                        or pending[0][1] != ki // 4
                        or pending[0][0] != h
                    ):
                        args = pending.pop(0)
                        mid = lambda a=args: do_apply(a)
                    attn_ki(h, ki, otp, mid=mid)
                    want = (ki + 1) * len(fill) // mt
                    while fi < want:
                        fill[fi]()
                        fi += 1
                    if ki % 4 == 3:
                        qc = ki // 4
                        recb = normalize_rec(h, qc, otp)
                        pending.append((h, qc, otp, recb))
                    if h == hl - 1 and cready and len(pending) <= 1:
                        phasec(cready.pop(0), (2, 3), out_d)
                while fi < len(fill):
                    fill[fi]()
                    fi += 1
            while pending:
                do_apply(pending.pop(0))
            while cready:
                phasec(cready.pop(0), (2, 3), out_d)

    nc.finalize()
    return nc


_NC_CACHE = {}


def _get_nc(key=(T, 512, 8, KD)):
    if key not in _NC_CACHE:
        _NC_CACHE[key] = build_nc(*key)
    return _NC_CACHE[key]


def make_in_maps(x, Wq, Wk, Wv, Wo, dl=512):
    in_maps = []
    for c in range(NCORES):
        b, g = c // 2, c % 2
        rows = slice(dl * g, dl * (g + 1))
        in_maps.append(
            {
                "xt": np.ascontiguousarray(x[b].T).astype(ml_dtypes.bfloat16),
                "wq": np.ascontiguousarray(Wq[rows, :].T).astype(ml_dtypes.bfloat16),
                "wk": np.ascontiguousarray(Wk[rows, :].T).astype(ml_dtypes.bfloat16),
                "wv": np.ascontiguousarray(Wv[rows, :].T).astype(ml_dtypes.bfloat16),
                "wo": np.ascontiguousarray(Wo[:, rows].T).astype(ml_dtypes.bfloat16),
            }
        )
    return in_maps


def run_spmd(x, Wq, Wk, Wv, Wo, trace=False):
    nc = _get_nc()
    in_maps = make_in_maps(x, Wq, Wk, Wv, Wo)
    res = run_bass_kernel_spmd(nc, in_maps, list(range(NCORES)), trace=trace)
    outs = [res.results[c]["out"] for c in range(NCORES)]
    outs2 = [res.results[c]["out2"] for c in range(NCORES)]
    final = np.stack(
        [
            outs[2 * b] + outs2[2 * b] + outs[2 * b + 1] + outs2[2 * b + 1]
            for b in range(B)
        ]
    )
    return final.astype(np.float32), res


def kernel(x, Wq, Wk, Wv, Wo):
    x = np.asarray(x, dtype=np.float32)
    Wq = np.asarray(Wq, dtype=np.float32)
    Wk = np.asarray(Wk, dtype=np.float32)
    Wv = np.asarray(Wv, dtype=np.float32)
    out, _ = run_spmd(x, Wq, Wk, Wv, np.asarray(Wo, dtype=np.float32))
    return out


# revision 22
# speedup vs baseline: 1.1052x; 1.0555x over previous
"""Multi-head causal attention (b=4, t=2048, k=1024, h=16) on 8 Trainium2 cores.

Sharding: core c = (batch b=c//2, head-group g=c%2). Each core computes one
batch x 8 heads; the two half-head partial outputs per batch are summed on
host.

Per-core kernel (bf16 matmul paths, fp32 PSUM), fully software-pipelined so
the tensor engine never idles (idle gaps also drop the PE clock 2.4->1.2GHz):
  - Q/K projections write per-head zero-PADDED tiles qtp/ktp[128, h, t]
    (head data in partitions 0:64, zeros in 64:128): the PE runs at half
    rate when contraction or stationary dims are < 128. Odd heads are
    placed via SBUF->SBUF DMA partition shift.
  - V is stored [128, ki, h, 128]: cols 0:64 = V, col 64 = ones (softmax
    denominator via the augmented PV matmul), 65:128 zeros.
  - Emission order interleaves projection matmul groups between attention
    chunks: K0/Q0 first, V tiles just-in-time inside head 0, K/Q tile g+1
    as filler during heads 2g-2..2g-1, output projection (phase C) m-tiles
    as filler inside head 7 as their ot columns complete.
  - Softmax: exp on ACT (bf16 out); reciprocal as exp(-ln d) on ACT (both
    funcs forced into one activation table => a single table load);
    denominator broadcast via a [1,128] ones matmul; normalize on DVE.
"""
import sys

sys.path.insert(0, "/opt/trn_rl_repo")

import numpy as np
import ml_dtypes

import concourse.bass as bass
import concourse.mybir as mybir
import concourse.tile as tile
from concourse import bacc
from concourse.bass_utils import run_bass_kernel_spmd
from concourse.masks import make_upper_triangular

# Force every ACT func (Exp/Ln/Copy) onto the one table that contains them
# all, so the table-load pass emits a single load instead of ping-ponging
# between exp-only and ln-only tables (1.3us per reload). Indices into
# act_info.json are preserved; only the candidacy of the other tables is
# hidden from the chooser.
_ORIG_GET_TABLES = bacc.get_activation_tables


def _single_table_get_activation_tables(arch):
    tabs = _ORIG_GET_TABLES(arch)
    if "natural_log_exp_and_others" not in tabs:
        return tabs
    return {
        name: (funcs if name == "natural_log_exp_and_others" else set())
        for name, funcs in tabs.items()
    }


bacc.get_activation_tables = _single_table_get_activation_tables

F32 = mybir.dt.float32
F32R = mybir.dt.float32r
BF16 = mybir.dt.bfloat16
EXP = mybir.ActivationFunctionType.Exp
LN = mybir.ActivationFunctionType.Ln

B, T, KD, NH, HS = 4, 2048, 1024, 16, 64
NCORES = 8


def build_nc(t=T, dl=512, hl=8, kd=KD):
    """One core's program: x.T [kd,t], per-group weights, partial out [t,kd]."""
    nk = kd // 128       # contraction tiles for projections
    mt = t // 128        # t tiles (also k-position tiles in attention)
    dt = dl // 128       # local-dim tiles
    nqc = t // 512       # q chunks
    scale = 1.0 / float(np.sqrt(kd))

    nc = bacc.Bacc("TRN2", target_bir_lowering=False, debug=False, num_devices=NCORES)
    xt_d = nc.dram_tensor("xt", [kd, t], BF16, kind="ExternalInput")
    wq_d = nc.dram_tensor("wq", [kd, dl], BF16, kind="ExternalInput")
    wk_d = nc.dram_tensor("wk", [kd, dl], BF16, kind="ExternalInput")
    wv_d = nc.dram_tensor("wv", [kd, dl], BF16, kind="ExternalInput")
    wo_d = nc.dram_tensor("wo", [dl, kd], BF16, kind="ExternalInput")
    out_d = nc.dram_tensor("out", [t, kd], F32, kind="ExternalOutput")
    out2_d = nc.dram_tensor("out2", [t, kd], F32, kind="ExternalOutput")

    with tile.TileContext(nc) as tc:
        with (
            tc.tile_pool(name="persist", bufs=1) as pp,
            tc.tile_pool(name="misc", bufs=1) as mp,
            tc.tile_pool(name="pa", bufs=1) as pa,
            tc.tile_pool(name="past", bufs=4) as past,
            tc.tile_pool(name="pbe", bufs=4) as pbe,
            tc.tile_pool(name="pbm", bufs=2) as pbm,
            tc.tile_pool(name="pco", bufs=2) as pco,
            tc.tile_pool(name="ps", bufs=1, space="PSUM") as ps,
        ):
            qtp = pp.tile([128, hl, t], BF16)   # per-head padded Q^T
            ktp = pp.tile([128, hl, t], BF16)   # per-head padded K^T
            v_s = pp.tile([128, mt, hl, 128], BF16)  # V | ones | zeros
            ot_s = pp.tile([128, dt, t], BF16)
            wo_s = pp.tile([128, dt, kd], BF16)
            mask_t = mp.tile([128, 128], BF16)
            ones_t = mp.tile([1, 128], BF16)
            ones128 = mp.tile([128, 128], BF16)
            xt_s = pa.tile([128, nk, t], BF16)
            wq_s = pa.tile([128, nk, dl], BF16)
            wk_s = pa.tile([128, nk, dl], BF16)
            wv_s = pa.tile([128, nk, dl], BF16)

            # ---------------- input DMAs (priority order) ----------------
            nc.sync.dma_start(
                wk_s[:, :, :], wk_d[:, :].rearrange("(n p) d -> p n d", p=128)
            )
            xt_r = xt_d[:, :].rearrange("(n p) t -> p n t", p=128)
            for k in range(nk):
                nc.sync.dma_start(xt_s[:, k, :], xt_r[:, k, :])
            nc.sync.dma_start(
                wq_s[:, :, :], wq_d[:, :].rearrange("(n p) d -> p n d", p=128)
            )
            nc.sync.dma_start(
                wv_s[:, :, :], wv_d[:, :].rearrange("(n p) d -> p n d", p=128)
            )
            nc.sync.dma_start(
                wo_s[:, :, :], wo_d[:, :].rearrange("(n p) o -> p n o", p=128)
            )

            # ---------------- constants + padding zeros ----------------
            make_upper_triangular(nc, mask_t[:, :], val=1.0, diag=True)
            nc.vector.memset(ones128[:, :], 1.0)
            nc.scalar.copy(ones_t[:, :], ones128[0:1, :])
            nc.scalar.copy(
                v_s[:, :, :, 64],
                ones128[:, 0 : mt * hl].rearrange("p (m h) -> p m h", m=mt),
            )
            # fine-grained zeroing in first-use order: head h's pad gates
            # only that head's scores; v pad per ki gates only that PV step.
            nc.gpsimd.memset(qtp[64:128, 0, :], 0.0)
            nc.gpsimd.memset(ktp[64:128, 0, :], 0.0)
            for ki in range(4):
                nc.gpsimd.memset(v_s[:, ki, :, 65:128], 0.0)
            for h in range(1, hl):
                nc.gpsimd.memset(qtp[64:128, h, :], 0.0)
                nc.gpsimd.memset(ktp[64:128, h, :], 0.0)
                if h < 4:
                    for ki in range(4 * h, 4 * h + 4):
                        nc.gpsimd.memset(v_s[:, ki, :, 65:128], 0.0)

            # ---------------- emission helpers ----------------
            def proj_v(m):
                psv = ps.tile([128, dl], F32, name=f"psv{m}", tag="proj", bufs=2)
                for k in range(nk):
                    nc.tensor.matmul(
                        psv[:, :],
                        xt_s[:, k, 128 * m : 128 * m + 128],
                        wv_s[:, k, :],
                        start=(k == 0),
                        stop=(k == nk - 1),
                    )
                nc.vector.tensor_copy(
                    v_s[:, m, :, 0:64],
                    psv[:, :].rearrange("p (h d) -> p h d", h=hl),
                )

            def proj_qk(w_s, o_s, pfx, m, n):
                # one 512-col group of Q or K dtile m (heads 2m, 2m+1)
                cols = slice(512 * n, 512 * n + 512)
                psq = ps.tile(
                    [128, 512], F32, name=f"ps{pfx}{m}_{n}", tag="proj", bufs=2
                )
                for k in range(nk):
                    nc.tensor.matmul(
                        psq[:, :],
                        w_s[:, k, 128 * m : 128 * m + 128],
                        xt_s[:, k, cols],
                        start=(k == 0),
                        stop=(k == nk - 1),
                    )
                nc.vector.tensor_copy(o_s[0:64, 2 * m, cols], psq[0:64, :])
                st = past.tile(
                    [128, 512], BF16, name=f"st{pfx}{m}_{n}", tag="stage"
                )
                nc.vector.tensor_copy(st[64:128, :], psq[64:128, :])
                nc.sync.dma_start(o_s[0:64, 2 * m + 1, cols], st[64:128, :])

            def score_pass(h, ki, clo):
                # scores for columns [max(128*ki, clo), clo+1024) into ONE
                # [128,1024] psum tile (2 banks), single exp (bf16 out)
                q0 = 128 * ki
                off = max(q0, clo)
                w = clo + 1024 - off
                stp = ps.tile(
                    [128, 1024], F32, name=f"st{h}_{ki}_{clo}", tag="st", bufs=2
                )
                o = off - clo
                for j0, j1 in ((o, 512), (max(o, 512), 1024)):
                    if j1 <= j0:
                        continue
                    nc.tensor.matmul(
                        stp[:, j0:j1],
                        ktp[:, h, q0 : q0 + 128],
                        qtp[:, h, clo + j0 : clo + j1],
                        start=True,
                        stop=True,
                    )
                ex = pbe.tile(
                    [128, 1024], BF16, name=f"ex{h}_{ki}_{clo}", tag="exp"
                )
                nc.scalar.activation(ex[:, :w], stp[:, o : o + w], EXP, scale=scale)
                if off == q0:
                    nc.vector.tensor_mul(ex[:, 0:128], ex[:, 0:128], mask_t[:, :])
                return ex

            def pv_pass(h, ki, clo, otp, ex):
                q0 = 128 * ki
                off = max(q0, clo)
                for qc in (clo // 512, clo // 512 + 1):
                    hi = 512 * (qc + 1)
                    lo = max(off, 512 * qc)
                    if hi <= lo:
                        continue
                    nc.tensor.matmul(
                        otp[qc][:, lo - 512 * qc : hi - 512 * qc],
                        v_s[:, ki, h, :],
                        ex[:, lo - off : hi - off],
                        start=(ki == 0),
                        stop=(ki == 4 * qc + 3),
                    )

            def attn_ki(h, ki, clo, otp, mid=None):
                ex = score_pass(h, ki, clo)
                if mid is not None:
                    mid()
                pv_pass(h, ki, clo, otp, ex)

            def normalize_rec(h, qc, otp):
                # 1/d = exp(-ln d) on ACT (no table swap). Emitted as soon
                # as otp[qc] stops accumulating; the tensor-side apply is
                # deferred so the bc matmul never waits on this ACT chain.
                rec = pbm.tile([1, 512], F32, name=f"rc{h}_{qc}", tag="rec")
                recb = pbm.tile([1, 512], BF16, name=f"rb{h}_{qc}", tag="recb")
                with nc.allow_low_precision(reason="softmax denom"):
                    nc.scalar.activation(rec[:, :], otp[qc][64:65, :], LN)
                    nc.scalar.activation(recb[:, :], rec[:, :], EXP, scale=-1.0)
                return recb

            def normalize_apply(h, qc, otp, recb):
                mh, ph = h // 2, 64 * (h % 2)
                bc = ps.tile([128, 512], F32, name=f"bc{h}_{qc}", tag="st", bufs=2)
                nc.tensor.matmul(
                    bc[:, :], ones_t[:, :], recb[:, :], start=True, stop=True
                )
                cols = slice(512 * qc, 512 * qc + 512)
                with nc.allow_low_precision(reason="softmax normalize"):
                    if ph == 0:
                        dst = ot_s[0:64, mh, cols]
                        nc.vector.tensor_copy(dst, otp[qc][0:64, :])
                        nc.vector.tensor_mul(dst, dst, bc[0:64, :])
                    else:
                        sc = pbm.tile(
                            [64, 512], BF16, name=f"sc{h}_{qc}", tag="scr"
                        )
                        nc.vector.tensor_copy(sc[:, :], otp[qc][0:64, :])
                        nc.vector.tensor_mul(sc[:, :], sc[:, :], bc[0:64, :])
                        nc.sync.dma_start(ot_s[64:128, mh, cols], sc[:, :])

            def phasec(m, ks, dst):
                ob = pco.tile([128, kd], F32, name=f"ob{ks[0]}_{m}", tag="ob")
                for n in range(kd // 512):
                    pso = ps.tile(
                        [128, 512], F32, name=f"pso{ks[0]}_{m}_{n}", tag="proj",
                        bufs=2,
                    )
                    for j, k in enumerate(ks):
                        nc.tensor.matmul(
                            pso[:, :],
                            ot_s[:, k, 128 * m : 128 * m + 128],
                            wo_s[:, k, 512 * n : 512 * n + 512],
                            start=(j == 0),
                            stop=(j == len(ks) - 1),
                        )
                    nc.vector.tensor_copy(ob[:, 512 * n : 512 * n + 512], pso[:, :])
                nc.sync.dma_start(dst[128 * m : 128 * m + 128, :], ob[:, :])

            # ---------------- emission schedule ----------------
            # Each head runs two column passes (cols 0:1024 over ki 0..7,
            # then 1024:2048 over ki 0..15): only 2 otp banks live at once,
            # freeing PSUM for [128,1024] score tiles (one exp per ki).
            # Head 0 prologue: only K0g0+Q0g0/g1 gate the first scores, so
            # the ACT exp chain starts as soon as x/wk/wq arrive.
            otp0 = {
                qc: ps.tile([128, 512], F32, name=f"otp0_{qc}", tag="ot", bufs=2)
                for qc in (0, 1)
            }
            proj_qk(wk_s, ktp, "k", 0, 0)
            proj_qk(wq_s, qtp, "q", 0, 0)
            proj_qk(wq_s, qtp, "q", 0, 1)
            ex00 = score_pass(0, 0, 0)
            proj_v(0)
            pv_pass(0, 0, 0, otp0, ex00)

            # filler projection groups: heads 2g,2g+1 need K/Q tile g,
            # emitted during heads 2g-2 and 2g-1; phase C first half
            # (heads 0..3, k-tiles 0,1) fills heads 4..6 into out2.
            filler = {h: [] for h in range(hl)}
            filler[0] = [
                (lambda: proj_qk(wq_s, qtp, "q", 0, 2)),
                (lambda: proj_qk(wq_s, qtp, "q", 0, 3)),
                (lambda: proj_qk(wk_s, ktp, "k", 0, 1)),
                (lambda: proj_qk(wk_s, ktp, "k", 0, 2)),
                (lambda: proj_qk(wk_s, ktp, "k", 0, 3)),
            ]
            for g in range(1, dt):
                filler[2 * g - 2] += [
                    (lambda gg=g, nn=n: proj_qk(wk_s, ktp, "k", gg, nn))
                    for n in range(nqc)
                ]
                filler[2 * g - 1] += [
                    (lambda gg=g, nn=n: proj_qk(wq_s, qtp, "q", gg, nn))
                    for n in range(nqc)
                ]
            filler[4] += [(lambda mm=m: phasec(mm, (0, 1), out2_d)) for m in range(4)]
            filler[6] = [(lambda mm=m: phasec(mm, (0, 1), out2_d)) for m in range(4, mt)]

            cready = []   # phase C second-half m-tiles (during h7)
            pending = []  # deferred normalize applies: (h, qc, otp, recb)

            def do_apply(args):
                ah, aqc = args[0], args[1]
                normalize_apply(*args)
                if ah == hl - 1:
                    cready.extend(range(4 * aqc, 4 * aqc + 4))

            for h in range(hl):
                fill = filler[h]
                fi = 0
                step = 0
                nsteps = 24  # 8 (pass 0) + 16 (pass 1)
                for pi, clo in enumerate((0, 1024)):
                    qcs = (clo // 512, clo // 512 + 1)
                    if h == 0 and pi == 0:
                        otp = otp0
                    else:
                        otp = {
                            qc: ps.tile(
                                [128, 512], F32,
                                name=f"otp{h}_{qc}", tag="ot", bufs=2,
                            )
                            for qc in qcs
                        }
                    kmax = 8 if pi == 0 else mt
                    for ki in range(1 if (h == 0 and pi == 0) else 0, kmax):
                        if h == 0 and (pi == 0 or ki >= 8):
                            proj_v(ki)
                        # pop a deferred apply between scores and PVs so the
                        # bc matmul's ACT dependency has had time to resolve
                        mid = None
                        if pending and (
                            len(pending) > 1
                            or pending[0][1] != ki // 4
                            or pending[0][0] != h
                        ):
                            args = pending.pop(0)
                            mid = lambda a=args: do_apply(a)
                        attn_ki(h, ki, clo, otp, mid=mid)
                        step += 1
                        want = step * len(fill) // nsteps
                        while fi < want:
                            fill[fi]()
                            fi += 1
                        if ki % 4 == 3 and ki // 4 in qcs:
                            qc = ki // 4
                            recb = normalize_rec(h, qc, otp)
                            pending.append((h, qc, otp, recb))
                        if h == hl - 1 and cready and len(pending) <= 1:
                            phasec(cready.pop(0), (2, 3), out_d)
                while fi < len(fill):
                    fill[fi]()
                    fi += 1
            while pending:
                do_apply(pending.pop(0))
            while cready:
                phasec(cready.pop(0), (2, 3), out_d)

    nc.finalize()
    return nc


_NC_CACHE = {}


def _get_nc(key=(T, 512, 8, KD)):
    if key not in _NC_CACHE:
        _NC_CACHE[key] = build_nc(*key)
    return _NC_CACHE[key]


def make_in_maps(x, Wq, Wk, Wv, Wo, dl=512):
    in_maps = []
    for c in range(NCORES):
        b, g = c // 2, c % 2
        rows = slice(dl * g, dl * (g + 1))
        in_maps.append(
            {
                "xt": np.ascontiguousarray(x[b].T).astype(ml_dtypes.bfloat16),
                "wq": np.ascontiguousarray(Wq[rows, :].T).astype(ml_dtypes.bfloat16),
                "wk": np.ascontiguousarray(Wk[rows, :].T).astype(ml_dtypes.bfloat16),
                "wv": np.ascontiguousarray(Wv[rows, :].T).astype(ml_dtypes.bfloat16),
                "wo": np.ascontiguousarray(Wo[:, rows].T).astype(ml_dtypes.bfloat16),
            }
        )
    return in_maps


def run_spmd(x, Wq, Wk, Wv, Wo, trace=False):
    nc = _get_nc()
    in_maps = make_in_maps(x, Wq, Wk, Wv, Wo)
    res = run_bass_kernel_spmd(nc, in_maps, list(range(NCORES)), trace=trace)
    outs = [res.results[c]["out"] for c in range(NCORES)]
    outs2 = [res.results[c]["out2"] for c in range(NCORES)]
    final = np.stack(
        [
            outs[2 * b] + outs2[2 * b] + outs[2 * b + 1] + outs2[2 * b + 1]
            for b in range(B)
        ]
    )
    return final.astype(np.float32), res


def kernel(x, Wq, Wk, Wv, Wo):
    x = np.asarray(x, dtype=np.float32)
    Wq = np.asarray(Wq, dtype=np.float32)
    Wk = np.asarray(Wk, dtype=np.float32)
    Wv = np.asarray(Wv, dtype=np.float32)
    out, _ = run_spmd(x, Wq, Wk, Wv, np.asarray(Wo, dtype=np.float32))
    return out


# revision 24
# speedup vs baseline: 1.2392x; 1.1213x over previous
"""Multi-head causal attention (b=4, t=2048, k=1024, h=16) on 8 Trainium2 cores.

Sharding: core c = (batch b=c//2, head-group g=c%2). Each core computes one
batch x 8 heads; the two half-head partial outputs per batch are summed on
host.

Per-core kernel (bf16 matmul paths, fp32 PSUM), fully software-pipelined so
the tensor engine never idles (idle gaps also drop the PE clock 2.4->1.2GHz):
  - Q/K projections write per-head zero-PADDED tiles qtp/ktp[128, h, t]
    (head data in partitions 0:64, zeros in 64:128): the PE runs at half
    rate when contraction or stationary dims are < 128. Odd heads are
    placed via SBUF->SBUF DMA partition shift.
  - V is stored [128, ki, h, 128]: cols 0:64 = V, col 64 = ones (softmax
    denominator via the augmented PV matmul), 65:128 zeros.
  - Emission order interleaves projection matmul groups between attention
    chunks: K0/Q0 first, V tiles just-in-time inside head 0, K/Q tile g+1
    as filler during heads 2g-2..2g-1, output projection (phase C) m-tiles
    as filler inside head 7 as their ot columns complete.
  - Softmax: exp on ACT (bf16 out); reciprocal as exp(-ln d) on ACT (both
    funcs forced into one activation table => a single table load);
    denominator broadcast via a [1,128] ones matmul; normalize on DVE.
"""
import sys

sys.path.insert(0, "/opt/trn_rl_repo")

import numpy as np
import ml_dtypes

import concourse.bass as bass
import concourse.mybir as mybir
import concourse.tile as tile
from concourse import bacc
from concourse.bass_utils import run_bass_kernel_spmd
from concourse.masks import make_upper_triangular

# Force every ACT func (Exp/Ln/Copy) onto the one table that contains them
# all, so the table-load pass emits a single load instead of ping-ponging
# between exp-only and ln-only tables (1.3us per reload). Indices into
# act_info.json are preserved; only the candidacy of the other tables is
# hidden from the chooser.
_ORIG_GET_TABLES = bacc.get_activation_tables


def _single_table_get_activation_tables(arch):
    tabs = _ORIG_GET_TABLES(arch)
    if "natural_log_exp_and_others" not in tabs:
        return tabs
    return {
        name: (funcs if name == "natural_log_exp_and_others" else set())
        for name, funcs in tabs.items()
    }


bacc.get_activation_tables = _single_table_get_activation_tables

F32 = mybir.dt.float32
F32R = mybir.dt.float32r
BF16 = mybir.dt.bfloat16
EXP = mybir.ActivationFunctionType.Exp
LN = mybir.ActivationFunctionType.Ln

B, T, KD, NH, HS = 4, 2048, 1024, 16, 64
NCORES = 8
USE_PBCAST = True


def build_nc(t=T, dl=512, hl=8, kd=KD):
    """One core's program: x.T [kd,t], per-group weights, partial out [t,kd]."""
    nk = kd // 128       # contraction tiles for projections
    mt = t // 128        # t tiles (also k-position tiles in attention)
    dt = dl // 128       # local-dim tiles
    nqc = t // 512       # q chunks
    scale = 1.0 / float(np.sqrt(kd))

    nc = bacc.Bacc("TRN2", target_bir_lowering=False, debug=False, num_devices=NCORES)
    xt_d = nc.dram_tensor("xt", [kd, t], BF16, kind="ExternalInput")
    wq_d = nc.dram_tensor("wq", [kd, dl], BF16, kind="ExternalInput")
    wk_d = nc.dram_tensor("wk", [kd, dl], BF16, kind="ExternalInput")
    wv_d = nc.dram_tensor("wv", [kd, dl], BF16, kind="ExternalInput")
    wo_d = nc.dram_tensor("wo", [dl, kd], BF16, kind="ExternalInput")
    out_d = nc.dram_tensor("out", [t, kd], F32, kind="ExternalOutput")
    out2_d = nc.dram_tensor("out2", [t, kd], F32, kind="ExternalOutput")

    with tile.TileContext(nc) as tc:
        with (
            tc.tile_pool(name="persist", bufs=1) as pp,
            tc.tile_pool(name="misc", bufs=1) as mp,
            tc.tile_pool(name="pa", bufs=1) as pa,
            tc.tile_pool(name="past", bufs=4) as past,
            tc.tile_pool(name="pbe", bufs=4) as pbe,
            tc.tile_pool(name="pbm", bufs=2) as pbm,
            tc.tile_pool(name="pco", bufs=2) as pco,
            tc.tile_pool(name="ps", bufs=1, space="PSUM") as ps,
        ):
            qtp = pp.tile([128, hl, t], BF16)   # per-head padded Q^T
            ktp = pp.tile([128, hl, t], BF16)   # per-head padded K^T
            v_s = pp.tile([128, mt, hl, 128], BF16)  # V | ones | zeros
            ot_s = pp.tile([128, dt, t], BF16)
            wo_s = pp.tile([128, dt, kd], BF16)
            mask_t = mp.tile([128, 128], BF16)
            ones_t = mp.tile([1, 128], BF16)
            ones128 = mp.tile([128, 128], BF16)
            xt_s = pa.tile([128, nk, t], BF16)
            wq_s = pa.tile([128, nk, dl], BF16)
            wk_s = pa.tile([128, nk, dl], BF16)
            wv_s = pa.tile([128, nk, dl], BF16)

            # ------------- input DMAs (criticality order) -------------
            # wk+wq first, then the x columns pass 0 needs, then wv, then
            # the rest of x, wo last (phase C is ~200us away).
            nc.sync.dma_start(
                wk_s[:, :, :], wk_d[:, :].rearrange("(n p) d -> p n d", p=128)
            )
            nc.sync.dma_start(
                wq_s[:, :, :], wq_d[:, :].rearrange("(n p) d -> p n d", p=128)
            )
            xt_r = xt_d[:, :].rearrange("(n p) t -> p n t", p=128)
            for k in range(nk):
                nc.sync.dma_start(xt_s[:, k, 0:1024], xt_r[:, k, 0:1024])
            nc.sync.dma_start(
                wv_s[:, :, :], wv_d[:, :].rearrange("(n p) d -> p n d", p=128)
            )
            for k in range(nk):
                nc.sync.dma_start(xt_s[:, k, 1024:2048], xt_r[:, k, 1024:2048])
            nc.sync.dma_start(
                wo_s[:, :, :], wo_d[:, :].rearrange("(n p) o -> p n o", p=128)
            )

            # ---------------- constants + padding zeros ----------------
            make_upper_triangular(nc, mask_t[:, :], val=1.0, diag=True)
            nc.vector.memset(ones128[:, :], 1.0)
            nc.scalar.copy(ones_t[:, :], ones128[0:1, :])
            nc.scalar.copy(
                v_s[:, :, :, 64],
                ones128[:, 0 : mt * hl].rearrange("p (m h) -> p m h", m=mt),
            )
            # fine-grained zeroing, scheduled so the gpsimd queue stays
            # just ahead of first use (it also runs the normalize
            # broadcasts later; a long memset backlog would stall them).
            for h in (0, 1):
                nc.gpsimd.memset(qtp[64:128, h, :], 0.0)
                nc.gpsimd.memset(ktp[64:128, h, :], 0.0)
            for ki in range(4):
                nc.gpsimd.memset(v_s[:, ki, :, 65:128], 0.0)

            def memset_pad(h):
                nc.gpsimd.memset(qtp[64:128, h, :], 0.0)
                nc.gpsimd.memset(ktp[64:128, h, :], 0.0)

            def memset_v(k0, k1):
                for ki in range(k0, k1):
                    nc.gpsimd.memset(v_s[:, ki, :, 65:128], 0.0)

            # ---------------- emission helpers ----------------
            def proj_v(m):
                psv = ps.tile([128, dl], F32, name=f"psv{m}", tag="proj", bufs=2)
                for k in range(nk):
                    nc.tensor.matmul(
                        psv[:, :],
                        xt_s[:, k, 128 * m : 128 * m + 128],
                        wv_s[:, k, :],
                        start=(k == 0),
                        stop=(k == nk - 1),
                    )
                nc.vector.tensor_copy(
                    v_s[:, m, :, 0:64],
                    psv[:, :].rearrange("p (h d) -> p h d", h=hl),
                )

            def proj_qk(w_s, o_s, pfx, m, n):
                # one 512-col group of Q or K dtile m (heads 2m, 2m+1)
                cols = slice(512 * n, 512 * n + 512)
                psq = ps.tile(
                    [128, 512], F32, name=f"ps{pfx}{m}_{n}", tag="proj", bufs=2
                )
                for k in range(nk):
                    nc.tensor.matmul(
                        psq[:, :],
                        w_s[:, k, 128 * m : 128 * m + 128],
                        xt_s[:, k, cols],
                        start=(k == 0),
                        stop=(k == nk - 1),
                    )
                nc.vector.tensor_copy(o_s[0:64, 2 * m, cols], psq[0:64, :])
                st = past.tile(
                    [128, 512], BF16, name=f"st{pfx}{m}_{n}", tag="stage"
                )
                nc.vector.tensor_copy(st[64:128, :], psq[64:128, :])
                nc.sync.dma_start(o_s[0:64, 2 * m + 1, cols], st[64:128, :])

            def score_pass(h, ki, clo):
                # scores for columns [max(128*ki, clo), clo+1024) into ONE
                # [128,1024] psum tile (2 banks), single exp (bf16 out)
                q0 = 128 * ki
                off = max(q0, clo)
                w = clo + 1024 - off
                stp = ps.tile(
                    [128, 1024], F32, name=f"st{h}_{ki}_{clo}", tag="st", bufs=2
                )
                o = off - clo
                for j0, j1 in ((o, 512), (max(o, 512), 1024)):
                    if j1 <= j0:
                        continue
                    nc.tensor.matmul(
                        stp[:, j0:j1],
                        ktp[:, h, q0 : q0 + 128],
                        qtp[:, h, clo + j0 : clo + j1],
                        start=True,
                        stop=True,
                    )
                ex = pbe.tile(
                    [128, 1024], BF16, name=f"ex{h}_{ki}_{clo}", tag="exp"
                )
                nc.scalar.activation(ex[:, :w], stp[:, o : o + w], EXP, scale=scale)
                if off == q0:
                    nc.vector.tensor_mul(ex[:, 0:128], ex[:, 0:128], mask_t[:, :])
                return ex

            def pv_pass(h, ki, clo, otp, ex):
                q0 = 128 * ki
                off = max(q0, clo)
                for qc in (clo // 512, clo // 512 + 1):
                    hi = 512 * (qc + 1)
                    lo = max(off, 512 * qc)
                    if hi <= lo:
                        continue
                    nc.tensor.matmul(
                        otp[qc][:, lo - 512 * qc : hi - 512 * qc],
                        v_s[:, ki, h, :],
                        ex[:, lo - off : hi - off],
                        start=(ki == 0),
                        stop=(ki == 4 * qc + 3),
                    )

            def attn_ki(h, ki, clo, otp, mid=None):
                ex = score_pass(h, ki, clo)
                if mid is not None:
                    mid()
                pv_pass(h, ki, clo, otp, ex)

            def normalize_rec(h, qc, otp):
                # 1/d = exp(-ln d) on ACT (no table swap). Emitted as soon
                # as otp[qc] stops accumulating; the tensor-side apply is
                # deferred so the bc matmul never waits on this ACT chain.
                rec = pbm.tile([1, 512], F32, name=f"rc{h}_{qc}", tag="rec")
                recb = pbm.tile([1, 512], BF16, name=f"rb{h}_{qc}", tag="recb")
                with nc.allow_low_precision(reason="softmax denom"):
                    nc.scalar.activation(rec[:, :], otp[qc][64:65, :], LN)
                    nc.scalar.activation(recb[:, :], rec[:, :], EXP, scale=-1.0)
                return recb

            def normalize_apply(h, qc, otp, recb):
                mh, ph = h // 2, 64 * (h % 2)
                if USE_PBCAST:
                    bcb = pbm.tile(
                        [64, 512], BF16, name=f"bc{h}_{qc}", tag="bcb", bufs=2
                    )
                    nc.gpsimd.partition_broadcast(bcb[:, :], recb[:, :])
                    bcv = bcb[:, :]
                else:
                    bc = ps.tile(
                        [128, 512], F32, name=f"bc{h}_{qc}", tag="st", bufs=2
                    )
                    nc.tensor.matmul(
                        bc[:, :], ones_t[:, :], recb[:, :], start=True, stop=True
                    )
                    bcv = bc[0:64, :]
                cols = slice(512 * qc, 512 * qc + 512)
                with nc.allow_low_precision(reason="softmax normalize"):
                    if ph == 0:
                        dst = ot_s[0:64, mh, cols]
                        nc.vector.tensor_copy(dst, otp[qc][0:64, :])
                        nc.vector.tensor_mul(dst, dst, bcv)
                    else:
                        sc = pbm.tile(
                            [64, 512], BF16, name=f"sc{h}_{qc}", tag="scr"
                        )
                        nc.vector.tensor_copy(sc[:, :], otp[qc][0:64, :])
                        nc.vector.tensor_mul(sc[:, :], sc[:, :], bcv)
                        nc.sync.dma_start(ot_s[64:128, mh, cols], sc[:, :])

            def phasec(m, ks, dst):
                ob = pco.tile([128, kd], F32, name=f"ob{ks[0]}_{m}", tag="ob")
                for n in range(kd // 512):
                    pso = ps.tile(
                        [128, 512], F32, name=f"pso{ks[0]}_{m}_{n}", tag="proj",
                        bufs=2,
                    )
                    for j, k in enumerate(ks):
                        nc.tensor.matmul(
                            pso[:, :],
                            ot_s[:, k, 128 * m : 128 * m + 128],
                            wo_s[:, k, 512 * n : 512 * n + 512],
                            start=(j == 0),
                            stop=(j == len(ks) - 1),
                        )
                    nc.vector.tensor_copy(ob[:, 512 * n : 512 * n + 512], pso[:, :])
                nc.sync.dma_start(dst[128 * m : 128 * m + 128, :], ob[:, :])

            # ---------------- emission schedule ----------------
            # Each head runs two column passes (cols 0:1024 over ki 0..7,
            # then 1024:2048 over ki 0..15): only 2 otp banks live at once,
            # freeing PSUM for [128,1024] score tiles (one exp per ki).
            # Head 0 prologue: only K0g0+Q0g0/g1 gate the first scores, so
            # the ACT exp chain starts as soon as x/wk/wq arrive.
            otp0 = {
                qc: ps.tile([128, 512], F32, name=f"otp0_{qc}", tag="ot", bufs=2)
                for qc in (0, 1)
            }
            proj_qk(wk_s, ktp, "k", 0, 0)
            proj_qk(wq_s, qtp, "q", 0, 0)
            proj_qk(wq_s, qtp, "q", 0, 1)
            ex00 = score_pass(0, 0, 0)
            proj_v(0)
            pv_pass(0, 0, 0, otp0, ex00)

            # filler projection groups: heads 2g,2g+1 need K/Q tile g,
            # emitted during heads 2g-2 and 2g-1; phase C first half
            # (heads 0..3, k-tiles 0,1) fills heads 4..6 into out2.
            filler = {h: [] for h in range(hl)}
            filler[0] = [
                (lambda: proj_qk(wq_s, qtp, "q", 0, 2)),
                (lambda: proj_qk(wq_s, qtp, "q", 0, 3)),
                (lambda: proj_qk(wk_s, ktp, "k", 0, 1)),
                (lambda: proj_qk(wk_s, ktp, "k", 0, 2)),
                (lambda: proj_qk(wk_s, ktp, "k", 0, 3)),
            ]
            for g in range(1, dt):
                filler[2 * g - 2] += [
                    (lambda gg=g, nn=n: proj_qk(wk_s, ktp, "k", gg, nn))
                    for n in range(nqc)
                ]
                filler[2 * g - 1] += [
                    (lambda gg=g, nn=n: proj_qk(wq_s, qtp, "q", gg, nn))
                    for n in range(nqc)
                ]
            filler[4] += [(lambda mm=m: phasec(mm, (0, 1), out2_d)) for m in range(4)]
            filler[6] = [(lambda mm=m: phasec(mm, (0, 1), out2_d)) for m in range(4, mt)]

            cready = []   # phase C second-half m-tiles (during h7)
            pending = []  # deferred normalize applies: (h, qc, otp, recb)

            def do_apply(args):
                ah, aqc = args[0], args[1]
                normalize_apply(*args)
                if ah == hl - 1:
                    cready.extend(range(4 * aqc, 4 * aqc + 4))

            for h in range(hl):
                if h + 2 < hl:
                    memset_pad(h + 2)
                if h == 0:
                    memset_v(4, 8)
                fill = filler[h]
                fi = 0
                step = 0
                nsteps = 24  # 8 (pass 0) + 16 (pass 1)
                for pi, clo in enumerate((0, 1024)):
                    qcs = (clo // 512, clo // 512 + 1)
                    if h == 0 and pi == 0:
                        otp = otp0
                    else:
                        otp = {
                            qc: ps.tile(
                                [128, 512], F32,
                                name=f"otp{h}_{qc}", tag="ot", bufs=2,
                            )
                            for qc in qcs
                        }
                    kmax = 8 if pi == 0 else mt
                    for ki in range(1 if (h == 0 and pi == 0) else 0, kmax):
                        if h == 0 and pi == 1 and ki in (0, 4):
                            memset_v(8 + ki, 12 + ki)
                        if h == 0 and (pi == 0 or ki >= 8):
                            proj_v(ki)
                        # pop a deferred apply between scores and PVs so the
                        # bc matmul's ACT dependency has had time to resolve
                        mid = None
                        if pending and (
                            len(pending) > 1
                            or pending[0][1] != ki // 4
                            or pending[0][0] != h
                        ):
                            args = pending.pop(0)
                            mid = lambda a=args: do_apply(a)
                        attn_ki(h, ki, clo, otp, mid=mid)
                        step += 1
                        want = step * len(fill) // nsteps
                        while fi < want:
                            fill[fi]()
                            fi += 1
                        if ki % 4 == 3 and ki // 4 in qcs:
                            qc = ki // 4
                            recb = normalize_rec(h, qc, otp)
                            pending.append((h, qc, otp, recb))
                        if h == hl - 1 and cready and len(pending) <= 1:
                            phasec(cready.pop(0), (2, 3), out_d)
                while fi < len(fill):
                    fill[fi]()
                    fi += 1
            while pending:
                do_apply(pending.pop(0))
            while cready:
                phasec(cready.pop(0), (2, 3), out_d)

    nc.finalize()
    return nc


_NC_CACHE = {}


def _get_nc(key=(T, 512, 8, KD)):
    if key not in _NC_CACHE:
        _NC_CACHE[key] = build_nc(*key)
    return _NC_CACHE[key]


def make_in_maps(x, Wq, Wk, Wv, Wo, dl=512):
    in_maps = []
    for c in range(NCORES):
        b, g = c // 2, c % 2
        rows = slice(dl * g, dl * (g + 1))
        in_maps.append(
            {
                "xt": np.ascontiguousarray(x[b].T).astype(ml_dtypes.bfloat16),
                "wq": np.ascontiguousarray(Wq[rows, :].T).astype(ml_dtypes.bfloat16),
                "wk": np.ascontiguousarray(Wk[rows, :].T).astype(ml_dtypes.bfloat16),
                "wv": np.ascontiguousarray(Wv[rows, :].T).astype(ml_dtypes.bfloat16),
                "wo": np.ascontiguousarray(Wo[:, rows].T).astype(ml_dtypes.bfloat16),
            }
        )
    return in_maps


def run_spmd(x, Wq, Wk, Wv, Wo, trace=False):
    nc = _get_nc()
    in_maps = make_in_maps(x, Wq, Wk, Wv, Wo)
    res = run_bass_kernel_spmd(nc, in_maps, list(range(NCORES)), trace=trace)
    outs = [res.results[c]["out"] for c in range(NCORES)]
    outs2 = [res.results[c]["out2"] for c in range(NCORES)]
    final = np.stack(
        [
            outs[2 * b] + outs2[2 * b] + outs[2 * b + 1] + outs2[2 * b + 1]
            for b in range(B)
        ]
    )
    return final.astype(np.float32), res


def kernel(x, Wq, Wk, Wv, Wo):
    x = np.asarray(x, dtype=np.float32)
    Wq = np.asarray(Wq, dtype=np.float32)
    Wk = np.asarray(Wk, dtype=np.float32)
    Wv = np.asarray(Wv, dtype=np.float32)
    out, _ = run_spmd(x, Wq, Wk, Wv, np.asarray(Wo, dtype=np.float32))
    return out


# revision 28
# speedup vs baseline: 1.2429x; 1.0030x over previous
"""Multi-head causal attention (b=4, t=2048, k=1024, h=16) on 8 Trainium2 cores.

Sharding: core c = (batch b=c//2, head-group g=c%2). Each core computes one
batch x 8 heads; the two half-head partial outputs per batch are summed on
host.

Per-core kernel (bf16 matmul paths, fp32 PSUM), fully software-pipelined so
the tensor engine never idles (idle gaps also drop the PE clock 2.4->1.2GHz):
  - Q/K projections write per-head zero-PADDED tiles qtp/ktp[128, h, t]
    (head data in partitions 0:64, zeros in 64:128): the PE runs at half
    rate when contraction or stationary dims are < 128. Odd heads are
    placed via SBUF->SBUF DMA partition shift.
  - V is stored [128, ki, h, 128]: cols 0:64 = V, col 64 = ones (softmax
    denominator via the augmented PV matmul), 65:128 zeros.
  - Emission order interleaves projection matmul groups between attention
    chunks: K0/Q0 first, V tiles just-in-time inside head 0, K/Q tile g+1
    as filler during heads 2g-2..2g-1, output projection (phase C) m-tiles
    as filler inside head 7 as their ot columns complete.
  - Softmax: exp on ACT (bf16 out); reciprocal as exp(-ln d) on ACT (both
    funcs forced into one activation table => a single table load);
    denominator broadcast via a [1,128] ones matmul; normalize on DVE.
"""
import sys

sys.path.insert(0, "/opt/trn_rl_repo")

import numpy as np
import ml_dtypes

import concourse.bass as bass
import concourse.mybir as mybir
import concourse.tile as tile
from concourse import bacc
from concourse.bass_utils import run_bass_kernel_spmd
from concourse.masks import make_upper_triangular

# Force every ACT func (Exp/Ln/Copy) onto the one table that contains them
# all, so the table-load pass emits a single load instead of ping-ponging
# between exp-only and ln-only tables (1.3us per reload). Indices into
# act_info.json are preserved; only the candidacy of the other tables is
# hidden from the chooser.
_ORIG_GET_TABLES = bacc.get_activation_tables


def _single_table_get_activation_tables(arch):
    tabs = _ORIG_GET_TABLES(arch)
    if "natural_log_exp_and_others" not in tabs:
        return tabs
    return {
        name: (funcs if name == "natural_log_exp_and_others" else set())
        for name, funcs in tabs.items()
    }


bacc.get_activation_tables = _single_table_get_activation_tables

F32 = mybir.dt.float32
F32R = mybir.dt.float32r
BF16 = mybir.dt.bfloat16
EXP = mybir.ActivationFunctionType.Exp
LN = mybir.ActivationFunctionType.Ln

B, T, KD, NH, HS = 4, 2048, 1024, 16, 64
NCORES = 8
USE_PBCAST = True


def build_nc(t=T, dl=512, hl=8, kd=KD):
    """One core's program: x.T [kd,t], per-group weights, partial out [t,kd]."""
    nk = kd // 128       # contraction tiles for projections
    mt = t // 128        # t tiles (also k-position tiles in attention)
    dt = dl // 128       # local-dim tiles
    nqc = t // 512       # q chunks
    scale = 1.0 / float(np.sqrt(kd))

    nc = bacc.Bacc("TRN2", target_bir_lowering=False, debug=False, num_devices=NCORES)
    xt_d = nc.dram_tensor("xt", [kd, t], BF16, kind="ExternalInput")
    wq_d = nc.dram_tensor("wq", [kd, dl], BF16, kind="ExternalInput")
    wk_d = nc.dram_tensor("wk", [kd, dl], BF16, kind="ExternalInput")
    wv_d = nc.dram_tensor("wv", [kd, dl], BF16, kind="ExternalInput")
    wo_d = nc.dram_tensor("wo", [dl, kd], BF16, kind="ExternalInput")
    out_d = nc.dram_tensor("out", [t, kd], F32, kind="ExternalOutput")
    out2_d = nc.dram_tensor("out2", [t, kd], F32, kind="ExternalOutput")

    with tile.TileContext(nc) as tc:
        with (
            tc.tile_pool(name="persist", bufs=1) as pp,
            tc.tile_pool(name="misc", bufs=1) as mp,
            tc.tile_pool(name="pa", bufs=1) as pa,
            tc.tile_pool(name="past", bufs=4) as past,
            tc.tile_pool(name="pbe", bufs=4) as pbe,
            tc.tile_pool(name="pbm", bufs=2) as pbm,
            tc.tile_pool(name="pco", bufs=2) as pco,
            tc.tile_pool(name="ps", bufs=1, space="PSUM") as ps,
        ):
            qtp = pp.tile([128, hl, t], BF16)   # per-head padded Q^T
            ktp = pp.tile([128, hl, t], BF16)   # per-head padded K^T
            v_s = pp.tile([128, mt, hl, 128], BF16)  # V | ones | zeros
            ot_s = pp.tile([128, dt, t], BF16)
            wo_s = pp.tile([128, dt, kd], BF16)
            mask_t = mp.tile([128, 128], BF16)
            ones_t = mp.tile([1, 128], BF16)
            ones128 = mp.tile([128, 128], BF16)
            xt_s = pa.tile([128, nk, t], BF16)
            wq_s = pa.tile([128, nk, dl], BF16)
            wk_s = pa.tile([128, nk, dl], BF16)
            wv_s = pa.tile([128, nk, dl], BF16)

            # ------------- input DMAs (criticality order) -------------
            # wk+wq first, then the x columns pass 0 needs, then wv, then
            # the rest of x, wo last (phase C is ~200us away).
            nc.sync.dma_start(
                wk_s[:, :, :], wk_d[:, :].rearrange("(n p) d -> p n d", p=128)
            )
            nc.sync.dma_start(
                wq_s[:, :, :], wq_d[:, :].rearrange("(n p) d -> p n d", p=128)
            )
            xt_r = xt_d[:, :].rearrange("(n p) t -> p n t", p=128)
            for k in range(nk):
                nc.sync.dma_start(xt_s[:, k, 0:1024], xt_r[:, k, 0:1024])
            nc.sync.dma_start(
                wv_s[:, :, :], wv_d[:, :].rearrange("(n p) d -> p n d", p=128)
            )
            for k in range(nk):
                nc.sync.dma_start(xt_s[:, k, 1024:2048], xt_r[:, k, 1024:2048])
            nc.sync.dma_start(
                wo_s[:, :, :], wo_d[:, :].rearrange("(n p) o -> p n o", p=128)
            )

            # ---------------- constants + padding zeros ----------------
            make_upper_triangular(nc, mask_t[:, :], val=1.0, diag=True)
            nc.vector.memset(ones128[:, :], 1.0)
            nc.scalar.copy(ones_t[:, :], ones128[0:1, :])
            nc.scalar.copy(
                v_s[:, :, :, 64],
                ones128[:, 0 : mt * hl].rearrange("p (m h) -> p m h", m=mt),
            )
            # fine-grained zeroing, scheduled so the gpsimd queue stays
            # just ahead of first use (it also runs the normalize
            # broadcasts later; a long memset backlog would stall them).
            for h in (0, 1):
                nc.gpsimd.memset(qtp[64:128, h, :], 0.0)
                nc.gpsimd.memset(ktp[64:128, h, :], 0.0)
            for ki in range(4):
                nc.gpsimd.memset(v_s[:, ki, :, 65:128], 0.0)

            def memset_pad(h):
                nc.gpsimd.memset(qtp[64:128, h, :], 0.0)
                nc.gpsimd.memset(ktp[64:128, h, :], 0.0)

            def memset_v(k0, k1):
                for ki in range(k0, k1):
                    nc.gpsimd.memset(v_s[:, ki, :, 65:128], 0.0)

            # ---------------- emission helpers ----------------
            def proj_v(m):
                psv = ps.tile([128, dl], F32, name=f"psv{m}", tag="proj", bufs=2)
                for k in range(nk):
                    nc.tensor.matmul(
                        psv[:, :],
                        xt_s[:, k, 128 * m : 128 * m + 128],
                        wv_s[:, k, :],
                        start=(k == 0),
                        stop=(k == nk - 1),
                    )
                nc.vector.tensor_copy(
                    v_s[:, m, :, 0:64],
                    psv[:, :].rearrange("p (h d) -> p h d", h=hl),
                )

            def proj_qk(w_s, o_s, pfx, m, n):
                # one 512-col group of Q or K dtile m (heads 2m, 2m+1)
                cols = slice(512 * n, 512 * n + 512)
                psq = ps.tile(
                    [128, 512], F32, name=f"ps{pfx}{m}_{n}", tag="proj", bufs=2
                )
                for k in range(nk):
                    nc.tensor.matmul(
                        psq[:, :],
                        w_s[:, k, 128 * m : 128 * m + 128],
                        xt_s[:, k, cols],
                        start=(k == 0),
                        stop=(k == nk - 1),
                    )
                nc.vector.tensor_copy(o_s[0:64, 2 * m, cols], psq[0:64, :])
                st = past.tile(
                    [128, 512], BF16, name=f"st{pfx}{m}_{n}", tag="stage"
                )
                nc.vector.tensor_copy(st[64:128, :], psq[64:128, :])
                nc.sync.dma_start(o_s[0:64, 2 * m + 1, cols], st[64:128, :])

            def score_pass(h, ki, clo):
                # scores for columns [max(128*ki, clo), clo+1024) into ONE
                # [128,1024] psum tile (2 banks), single exp (bf16 out)
                q0 = 128 * ki
                off = max(q0, clo)
                w = clo + 1024 - off
                stp = ps.tile(
                    [128, 1024], F32, name=f"st{h}_{ki}_{clo}", tag="st", bufs=2
                )
                o = off - clo
                for j0, j1 in ((o, 512), (max(o, 512), 1024)):
                    if j1 <= j0:
                        continue
                    nc.tensor.matmul(
                        stp[:, j0:j1],
                        ktp[:, h, q0 : q0 + 128],
                        qtp[:, h, clo + j0 : clo + j1],
                        start=True,
                        stop=True,
                    )
                ex = pbe.tile(
                    [128, 1024], BF16, name=f"ex{h}_{ki}_{clo}", tag="exp"
                )
                nc.scalar.activation(ex[:, :w], stp[:, o : o + w], EXP, scale=scale)
                if off == q0:
                    nc.vector.tensor_mul(ex[:, 0:128], ex[:, 0:128], mask_t[:, :])
                return ex

            def pv_pass(h, ki, clo, otp, ex):
                q0 = 128 * ki
                off = max(q0, clo)
                for qc in (clo // 512, clo // 512 + 1):
                    hi = 512 * (qc + 1)
                    lo = max(off, 512 * qc)
                    if hi <= lo:
                        continue
                    nc.tensor.matmul(
                        otp[qc][:, lo - 512 * qc : hi - 512 * qc],
                        v_s[:, ki, h, :],
                        ex[:, lo - off : hi - off],
                        start=(ki == 0),
                        stop=(ki == 4 * qc + 3),
                    )

            def attn_ki(h, ki, clo, otp, mid=None):
                ex = score_pass(h, ki, clo)
                if mid is not None:
                    mid()
                pv_pass(h, ki, clo, otp, ex)

            def normalize_rec(h, qc, otp):
                # 1/d = exp(-ln d) on ACT (no table swap). Emitted as soon
                # as otp[qc] stops accumulating; the tensor-side apply is
                # deferred so the bc matmul never waits on this ACT chain.
                rec = pbm.tile([1, 512], F32, name=f"rc{h}_{qc}", tag="rec", bufs=1)
                recb = pbm.tile(
                    [1, 512], BF16, name=f"rb{h}_{qc}", tag="recb", bufs=1
                )
                with nc.allow_low_precision(reason="softmax denom"):
                    nc.scalar.activation(rec[:, :], otp[qc][64:65, :], LN)
                    nc.scalar.activation(recb[:, :], rec[:, :], EXP, scale=-1.0)
                return recb

            def normalize_apply(h, qc, otp, recb):
                mh, ph = h // 2, 64 * (h % 2)
                if USE_PBCAST:
                    bcb = pbm.tile(
                        [64, 512], BF16, name=f"bc{h}_{qc}", tag="bcb", bufs=2
                    )
                    nc.gpsimd.partition_broadcast(bcb[:, :], recb[:, :])
                    bcv = bcb[:, :]
                else:
                    bc = ps.tile(
                        [128, 512], F32, name=f"bc{h}_{qc}", tag="st", bufs=2
                    )
                    nc.tensor.matmul(
                        bc[:, :], ones_t[:, :], recb[:, :], start=True, stop=True
                    )
                    bcv = bc[0:64, :]
                cols = slice(512 * qc, 512 * qc + 512)
                with nc.allow_low_precision(reason="softmax normalize"):
                    if ph == 0:
                        dst = ot_s[0:64, mh, cols]
                        nc.vector.tensor_copy(dst, otp[qc][0:64, :])
                        nc.vector.tensor_mul(dst, dst, bcv)
                    else:
                        sc = pbm.tile(
                            [64, 512], BF16, name=f"sc{h}_{qc}", tag="scr"
                        )
                        nc.vector.tensor_copy(sc[:, :], otp[qc][0:64, :])
                        nc.vector.tensor_mul(sc[:, :], sc[:, :], bcv)
                        nc.sync.dma_start(ot_s[64:128, mh, cols], sc[:, :])

            def phasec(m, ks, dst):
                ob = pco.tile([128, kd], F32, name=f"ob{ks[0]}_{m}", tag="ob")
                for n in range(kd // 512):
                    pso = ps.tile(
                        [128, 512], F32, name=f"pso{ks[0]}_{m}_{n}", tag="proj",
                        bufs=2,
                    )
                    for j, k in enumerate(ks):
                        nc.tensor.matmul(
                            pso[:, :],
                            ot_s[:, k, 128 * m : 128 * m + 128],
                            wo_s[:, k, 512 * n : 512 * n + 512],
                            start=(j == 0),
                            stop=(j == len(ks) - 1),
                        )
                    nc.vector.tensor_copy(ob[:, 512 * n : 512 * n + 512], pso[:, :])
                nc.sync.dma_start(dst[128 * m : 128 * m + 128, :], ob[:, :])

            # ---------------- emission schedule ----------------
            # Each head runs two column passes (cols 0:1024 over ki 0..7,
            # then 1024:2048 over ki 0..15): only 2 otp banks live at once,
            # freeing PSUM for [128,1024] score tiles (one exp per ki).
            # Head 0 prologue: only K0g0+Q0g0/g1 gate the first scores, so
            # the ACT exp chain starts as soon as x/wk/wq arrive.
            otp0 = {
                qc: ps.tile([128, 512], F32, name=f"otp0_{qc}", tag="ot", bufs=2)
                for qc in (0, 1)
            }
            proj_qk(wk_s, ktp, "k", 0, 0)
            proj_qk(wq_s, qtp, "q", 0, 0)
            proj_qk(wq_s, qtp, "q", 0, 1)
            ex00 = score_pass(0, 0, 0)
            proj_v(0)
            pv_pass(0, 0, 0, otp0, ex00)

            # filler projection groups: heads 2g,2g+1 need K/Q tile g,
            # emitted during heads 2g-2 and 2g-1; phase C first half
            # (heads 0..3, k-tiles 0,1) fills heads 4..6 into out2.
            filler = {h: [] for h in range(hl)}
            filler[0] = [
                (lambda: proj_qk(wq_s, qtp, "q", 0, 2)),
                (lambda: proj_qk(wq_s, qtp, "q", 0, 3)),
                (lambda: proj_qk(wk_s, ktp, "k", 0, 1)),
                (lambda: proj_qk(wk_s, ktp, "k", 0, 2)),
                (lambda: proj_qk(wk_s, ktp, "k", 0, 3)),
            ]
            for g in range(1, dt):
                filler[2 * g - 2] += [
                    (lambda gg=g, nn=n: proj_qk(wk_s, ktp, "k", gg, nn))
                    for n in range(nqc)
                ]
                filler[2 * g - 1] += [
                    (lambda gg=g, nn=n: proj_qk(wq_s, qtp, "q", gg, nn))
                    for n in range(nqc)
                ]
            filler[4] += [(lambda mm=m: phasec(mm, (0, 1), out2_d)) for m in range(4)]
            filler[6] = [(lambda mm=m: phasec(mm, (0, 1), out2_d)) for m in range(4, mt)]

            cready = []   # phase C second-half m-tiles (during h7)
            pending = []  # deferred normalize applies: (h, qc, otp, recb)

            def do_apply(args):
                ah, aqc = args[0], args[1]
                normalize_apply(*args)
                if ah == hl - 1:
                    cready.extend(range(4 * aqc, 4 * aqc + 4))

            for h in range(hl):
                if h + 2 < hl:
                    memset_pad(h + 2)
                if h == 0:
                    memset_v(4, 8)
                fill = filler[h]
                fi = 0
                step = 0
                nsteps = 24  # 8 (pass 0) + 16 (pass 1)
                for pi, clo in enumerate((0, 1024)):
                    qcs = (clo // 512, clo // 512 + 1)
                    if h == 0 and pi == 0:
                        otp = otp0
                    else:
                        otp = {
                            qc: ps.tile(
                                [128, 512], F32,
                                name=f"otp{h}_{qc}", tag="ot", bufs=2,
                            )
                            for qc in qcs
                        }
                    kmax = 8 if pi == 0 else mt
                    for ki in range(1 if (h == 0 and pi == 0) else 0, kmax):
                        if h == 0 and pi == 1 and ki in (0, 4):
                            memset_v(8 + ki, 12 + ki)
                        if h == 0 and (pi == 0 or ki >= 8):
                            proj_v(ki)
                        # pop a deferred apply between scores and PVs so the
                        # bc matmul's ACT dependency has had time to resolve
                        mid = None
                        if pending and (
                            len(pending) > 1
                            or pending[0][1] != ki // 4
                            or pending[0][0] != h
                        ):
                            args = pending.pop(0)
                            mid = lambda a=args: do_apply(a)
                        attn_ki(h, ki, clo, otp, mid=mid)
                        step += 1
                        want = step * len(fill) // nsteps
                        while fi < want:
                            fill[fi]()
                            fi += 1
                        if ki % 4 == 3 and ki // 4 in qcs:
                            qc = ki // 4
                            recb = normalize_rec(h, qc, otp)
                            pending.append((h, qc, otp, recb))
                        if h == hl - 1 and cready and len(pending) <= 1:
                            phasec(cready.pop(0), (2, 3), out_d)
                while fi < len(fill):
                    fill[fi]()
                    fi += 1
            while pending:
                do_apply(pending.pop(0))
            while cready:
                phasec(cready.pop(0), (2, 3), out_d)

    nc.finalize()
    return nc


_NC_CACHE = {}


def _get_nc(key=(T, 512, 8, KD)):
    if key not in _NC_CACHE:
        _NC_CACHE[key] = build_nc(*key)
    return _NC_CACHE[key]


def make_in_maps(x, Wq, Wk, Wv, Wo, dl=512):
    in_maps = []
    for c in range(NCORES):
        b, g = c // 2, c % 2
        rows = slice(dl * g, dl * (g + 1))
        in_maps.append(
            {
                "xt": np.ascontiguousarray(x[b].T).astype(ml_dtypes.bfloat16),
                "wq": np.ascontiguousarray(Wq[rows, :].T).astype(ml_dtypes.bfloat16),
                "wk": np.ascontiguousarray(Wk[rows, :].T).astype(ml_dtypes.bfloat16),
                "wv": np.ascontiguousarray(Wv[rows, :].T).astype(ml_dtypes.bfloat16),
                "wo": np.ascontiguousarray(Wo[:, rows].T).astype(ml_dtypes.bfloat16),
            }
        )
    return in_maps


def run_spmd(x, Wq, Wk, Wv, Wo, trace=False):
    nc = _get_nc()
    in_maps = make_in_maps(x, Wq, Wk, Wv, Wo)
    res = run_bass_kernel_spmd(nc, in_maps, list(range(NCORES)), trace=trace)
    outs = [res.results[c]["out"] for c in range(NCORES)]
    outs2 = [res.results[c]["out2"] for c in range(NCORES)]
    final = np.stack(
        [
            outs[2 * b] + outs2[2 * b] + outs[2 * b + 1] + outs2[2 * b + 1]
            for b in range(B)
        ]
    )
    return final.astype(np.float32), res


def kernel(x, Wq, Wk, Wv, Wo):
    x = np.asarray(x, dtype=np.float32)
    Wq = np.asarray(Wq, dtype=np.float32)
    Wk = np.asarray(Wk, dtype=np.float32)
    Wv = np.asarray(Wv, dtype=np.float32)
    out, _ = run_spmd(x, Wq, Wk, Wv, np.asarray(Wo, dtype=np.float32))
    return out
